# revision 13
# baseline (speedup 1.0000x reference)
"""AffineEdgeAttention Trainium2 kernel.

out[b, i, j] = head[b, i] . w_h + dep[b, j] . w_d + edge_b
with w_h = edge_W[0, :D], w_d = edge_W[0, D:].

Sharding: data-parallel over batch; 16 batches / 8 cores = 2 per core.

Per core (memory-bound, ~20.75 MiB of HBM traffic ~= 58 us at 358 GB/s):
  - inputs stream in as contiguous 768KB chunk-pair tiles [128, 2, 768]
    on the sync HWDGE ring; outputs stream back on the same ring as 1 MiB
    stores, so the ring stays saturated end to end.
  - w / b are broadcast to all 128 partitions via K=1 ones-matmuls on the
    otherwise-idle PE (a stride-0 DMA broadcast costs ~8 us, PE ~2 us).
  - s_d chunk k: elementwise *w_d (DVE/GpSimd) + free-axis reduce
    (ACT accum / DVE) -> sd[:, k]; then one stationary-broadcast matmul
    (lhsT = sd column with free-stride 0, rhs = identity) transposes AND
    broadcasts it into PSUM [128, k*128:(k+1)*128] - no scatter DMA.
  - sdb_sb = PSUM + edge_b in one ACT op; every output chunk is then a
    single broadcast-add (sdb_sb + s_h[:, c]) split across DVE (2x mode
    from SBUF) and ACT, written into [128, 2, 1024] pair tiles.
"""

import sys

import numpy as np

for _p in ("/opt/trn_rl_repo", "/root/.axon_site/_ro/trn_rl_repo"):
    if _p not in sys.path:
        sys.path.insert(0, _p)

import concourse.bacc as bacc
import concourse.bass as bass
import concourse.tile as tile
from concourse import mybir
from concourse.bass_utils import run_bass_kernel_spmd

B, S, D = 16, 1024, 768
N_CORES = 8
BPC = B // N_CORES  # batches per core
P = 128
C = S // P  # 8 row-chunks of 128
NPAIR = C // 2  # 4 chunk-pair tiles per tensor per batch

F32 = mybir.dt.float32

# engine assignment per chunk index 0..7 ("V"=vector, "A"=scalar, "G"=gpsimd)
# DVE leads the dep chain (it gates the broadcast); GpSimd takes the tail
# chunks whose pair-tiles land later anyway.
DEP_MULT_ENG = ["V", "V", "V", "V", "V", "V", "G", "G"]
DEP_RED_ENG = ["A", "A", "A", "A", "V", "A", "A", "V"]
HEAD_MULT_ENG = ["G", "V", "G", "V", "V", "G", "V", "V"]
HEAD_RED_ENG = ["A", "V", "A", "V", "A", "V", "A", "A"]
OUT_ENG = ["V", "A", "V", "V", "A", "V", "A", "V"]


def build_program() -> bass.Bass:
    nc = bacc.Bacc("TRN2", target_bir_lowering=False, debug=False)
    head = nc.dram_tensor("head", [BPC, S, D], F32, kind="ExternalInput").ap()
    dep = nc.dram_tensor("dep", [BPC, S, D], F32, kind="ExternalInput").ap()
    w = nc.dram_tensor("edge_W", [1, 2 * D], F32, kind="ExternalInput").ap()
    b = nc.dram_tensor("edge_b", [1], F32, kind="ExternalInput").ap()
    out = nc.dram_tensor("out", [BPC, S, S], F32, kind="ExternalOutput").ap()

    # [b, t, p, c, d]: chunk-pair t, intra-pair c; rows (2t+c)*128+p
    head_v = head.rearrange("b (t c p) d -> b t p c d", c=2, p=P)
    dep_v = dep.rearrange("b (t c p) d -> b t p c d", c=2, p=P)
    # output pair view: row = t*256 + c*128 + p, flatten (p, c, j)
    out_v = out.rearrange("b (t c p) j -> b t p c j", c=2, p=P)

    with tile.TileContext(nc) as tc:
        with (
            tc.tile_pool(name="singles", bufs=1) as singles,
            tc.tile_pool(name="loads", bufs=2 * NPAIR) as loads,
            tc.tile_pool(name="svec", bufs=2) as svec,
            tc.tile_pool(name="scratch", bufs=3) as scratch,
            tc.tile_pool(name="bcast", bufs=2) as bcast,
            tc.tile_pool(name="outs", bufs=4) as outs,
            tc.tile_pool(name="psum", bufs=2, space="PSUM") as psum,
            tc.tile_pool(name="psinit", bufs=1, space="PSUM") as psinit,
        ):
            # ---- constants: identity, ones, w/b broadcast via PE ----
            iota_f = singles.tile([P, P], F32)
            nc.gpsimd.iota(
                iota_f, [[1, P]], channel_multiplier=0,
                allow_small_or_imprecise_dtypes=True,
            )
            iota_p = singles.tile([P, 1], F32)
            nc.gpsimd.iota(
                iota_p, [[0, 1]], channel_multiplier=1,
                allow_small_or_imprecise_dtypes=True,
            )
            ident = singles.tile([P, P], F32)
            nc.vector.tensor_scalar(
                out=ident, in0=iota_f, scalar1=iota_p, scalar2=None,
                op0=mybir.AluOpType.is_equal,
            )
            ones = singles.tile([1, P], F32)
            nc.vector.memset(ones, 1.0)

            # tiny loads go first on the fast sync HWDGE ring (a SWDGE
            # gpsimd load costs ~4.5us end-to-end and gated everything)
            w_row = singles.tile([1, 2 * D], F32)
            nc.sync.dma_start(out=w_row, in_=w)
            b_row = singles.tile([1, 1], F32)
            nc.sync.dma_start(out=b_row, in_=b[None, :])
            psw = psinit.tile([P, 2 * D], F32)
            for k in range(3):
                nc.tensor.matmul(
                    psw[:, k * 512 : (k + 1) * 512],
                    lhsT=ones,
                    rhs=w_row[:, k * 512 : (k + 1) * 512],
                    start=True,
                    stop=True,
                )
            wt = singles.tile([P, 2 * D], F32)
            nc.scalar.copy(out=wt, in_=psw)
            psb = psinit.tile([P, 1], F32)
            nc.tensor.matmul(psb, lhsT=ones, rhs=b_row, start=True, stop=True)
            bt = singles.tile([P, 1], F32)
            nc.vector.tensor_copy(bt, psb)

            def eng(name):
                return {"V": nc.vector, "A": nc.scalar, "G": nc.gpsimd}[name]

            def reduce_to(engine, dst, prod):
                if engine == "A":
                    trash = scratch.tile([P, D], F32, tag="trash")
                    nc.scalar.activation(
                        out=trash,
                        in_=prod,
                        func=mybir.ActivationFunctionType.Copy,
                        accum_out=dst,
                    )
                else:
                    nc.vector.reduce_sum(dst, prod, axis=mybir.AxisListType.X)

            for bi in range(BPC):
                # ---- loads: dep first (it gates the broadcast row) ----
                dep_t = []
                for t in range(NPAIR):
                    dt_ = loads.tile([P, 2, D], F32, tag="dep")
                    nc.sync.dma_start(out=dt_, in_=dep_v[bi, t])
                    dep_t.append(dt_)
                head_t = []
                for t in range(NPAIR):
                    ht = loads.tile([P, 2, D], F32, tag="head")
                    nc.sync.dma_start(out=ht, in_=head_v[bi, t])
                    head_t.append(ht)

                # ---- s_d chunks -> stationary-broadcast matmuls into PSUM ----
                sd = svec.tile([P, C], F32, tag="sd")
                ps = psum.tile([P, S], F32, tag="ps")
                for k in range(C):
                    src = dep_t[k // 2][:, k % 2, :]
                    prod = scratch.tile([P, D], F32, tag="prod")
                    eng(DEP_MULT_ENG[k]).tensor_mul(prod, src, wt[:, D : 2 * D])
                    reduce_to(DEP_RED_ENG[k], sd[:, k : k + 1], prod)
                    nc.tensor.matmul(
                        ps[:, k * P : (k + 1) * P],
                        lhsT=sd[:, k : k + 1].broadcast_to((P, P)),
                        rhs=ident,
                        start=True,
                        stop=True,
                    )
                # one SBUF copy of the broadcast row, with edge_b folded in;
                # DVE-side adds then run in 2x perf mode (SBUF source)
                sdb_sb = bcast.tile([P, S], F32, tag="sdbsb")
                nc.scalar.add(out=sdb_sb, in_=ps, add=bt)

                # ---- s_h chunks + output chunks ----
                sh = svec.tile([P, C], F32, tag="sh")
                for t in range(NPAIR):
                    ot = outs.tile([P, 2, S], F32, tag="ot")
                    for i in range(2):
                        c = 2 * t + i
                        src = head_t[t][:, i, :]
                        prod = scratch.tile([P, D], F32, tag="prod")
                        eng(HEAD_MULT_ENG[c]).tensor_mul(prod, src, wt[:, 0:D])
                        reduce_to(HEAD_RED_ENG[c], sh[:, c : c + 1], prod)
                        if OUT_ENG[c] == "A":
                            nc.scalar.add(
                                out=ot[:, i, :], in_=sdb_sb, add=sh[:, c : c + 1]
                            )
                        else:
                            nc.vector.tensor_scalar_add(
                                ot[:, i, :], sdb_sb, sh[:, c : c + 1]
                            )
                    nc.sync.dma_start(out=out_v[bi, t], in_=ot)
    nc.compile()
    return nc


def kernel(head, dep, edge_W, edge_b, _trace=False):
    nc = build_program()
    in_maps = []
    for k in range(N_CORES):
        in_maps.append(
            {
                "head": np.ascontiguousarray(head[k * BPC : (k + 1) * BPC]),
                "dep": np.ascontiguousarray(dep[k * BPC : (k + 1) * BPC]),
                "edge_W": np.ascontiguousarray(edge_W),
                "edge_b": np.ascontiguousarray(edge_b),
            }
        )
    res = run_bass_kernel_spmd(nc, in_maps, core_ids=list(range(N_CORES)), trace=_trace)
    out = np.concatenate([r["out"] for r in res.results], axis=0)
    if _trace:
        return out, res
    return out


if __name__ == "__main__":
    rng = np.random.default_rng(0)
    head = rng.standard_normal((B, S, D), dtype=np.float32)
    dep = rng.standard_normal((B, S, D), dtype=np.float32)
    edge_W = rng.standard_normal((1, 2 * D), dtype=np.float32)
    edge_b = rng.standard_normal((1,), dtype=np.float32)
    out = kernel(head, dep, edge_W, edge_b)
    ref = (
        head @ edge_W[0, :D]
    )[:, :, None] + (dep @ edge_W[0, D:])[:, None, :] + edge_b[0]
    err = np.abs(out - ref).max() / np.abs(ref).max()
    print("max rel err:", err)


# revision 16
# speedup vs baseline: 1.1598x; 1.1598x over previous
"""AffineEdgeAttention Trainium2 kernel.

out[b, i, j] = head[b, i] . w_h + dep[b, j] . w_d + edge_b
with w_h = edge_W[0, :D], w_d = edge_W[0, D:].

Sharding: data-parallel over batch; 16 batches / 8 cores = 2 per core.

Per core (memory-bound, ~20.75 MiB of HBM traffic ~= 58 us at 358 GB/s):
  - inputs stream in as contiguous 768KB chunk-pair tiles [128, 2, 768]
    on the sync HWDGE ring; outputs stream back on the same ring as 1 MiB
    stores, so the ring stays saturated end to end.
  - w / b are broadcast to all 128 partitions via K=1 ones-matmuls on the
    otherwise-idle PE (a stride-0 DMA broadcast costs ~8 us, PE ~2 us).
  - s_d chunk k: elementwise *w_d (DVE/GpSimd) + free-axis reduce
    (ACT accum / DVE) -> sd[:, k]; then one stationary-broadcast matmul
    (lhsT = sd column with free-stride 0, rhs = identity) transposes AND
    broadcasts it into PSUM [128, k*128:(k+1)*128] - no scatter DMA.
  - sdb_sb = PSUM + edge_b in one ACT op; every output chunk is then a
    single broadcast-add (sdb_sb + s_h[:, c]) split across DVE (2x mode
    from SBUF) and ACT, written into [128, 2, 1024] pair tiles.
"""

import sys

import numpy as np

for _p in ("/opt/trn_rl_repo", "/root/.axon_site/_ro/trn_rl_repo"):
    if _p not in sys.path:
        sys.path.insert(0, _p)

import concourse.bacc as bacc
import concourse.bass as bass
import concourse.tile as tile
from concourse import mybir
from concourse.bass_utils import run_bass_kernel_spmd

B, S, D = 16, 1024, 768
N_CORES = 8
BPC = B // N_CORES  # batches per core
P = 128
C = S // P  # 8 row-chunks of 128
NPAIR = C // 2  # 4 chunk-pair tiles per tensor per batch

F32 = mybir.dt.float32

# engine assignment per chunk index 0..7 ("V"=vector, "A"=scalar, "G"=gpsimd)
# DVE leads the dep chain (it gates the broadcast); GpSimd takes the tail
# chunks whose pair-tiles land later anyway.
DEP_MULT_ENG = ["V", "V", "V", "V", "V", "V", "G", "G"]
DEP_RED_ENG = ["A", "V", "A", "V", "A", "V", "A", "A"]
HEAD_MULT_ENG = ["G", "V", "G", "V", "G", "V", "V", "V"]
HEAD_RED_ENG = ["A", "V", "A", "V", "A", "A", "V", "A"]
OUT_ENG = ["V", "A", "V", "A", "V", "A", "V", "V"]


def build_program() -> bass.Bass:
    nc = bacc.Bacc("TRN2", target_bir_lowering=False, debug=False)
    head = nc.dram_tensor("head", [BPC, S, D], F32, kind="ExternalInput").ap()
    dep = nc.dram_tensor("dep", [BPC, S, D], F32, kind="ExternalInput").ap()
    w = nc.dram_tensor("edge_W", [1, 2 * D], F32, kind="ExternalInput").ap()
    b = nc.dram_tensor("edge_b", [1], F32, kind="ExternalInput").ap()
    out = nc.dram_tensor("out", [BPC, S, S], F32, kind="ExternalOutput").ap()

    # [b, t, p, c, d]: chunk-pair t, intra-pair c; rows (2t+c)*128+p
    head_v = head.rearrange("b (t c p) d -> b t p c d", c=2, p=P)
    dep_v = dep.rearrange("b (t c p) d -> b t p c d", c=2, p=P)
    # output pair view: row = t*256 + c*128 + p, flatten (p, c, j)
    out_v = out.rearrange("b (t c p) j -> b t p c j", c=2, p=P)

    with tile.TileContext(nc) as tc:
        with (
            tc.tile_pool(name="singles", bufs=1) as singles,
            tc.tile_pool(name="loads", bufs=2 * NPAIR) as loads,
            tc.tile_pool(name="svec", bufs=2) as svec,
            tc.tile_pool(name="scratch", bufs=3) as scratch,
            tc.tile_pool(name="bcast", bufs=2) as bcast,
            tc.tile_pool(name="outs", bufs=4) as outs,
            tc.tile_pool(name="psum", bufs=2, space="PSUM") as psum,
        ):
            # ---- constants: identity, ones, w/b broadcast via PE ----
            iota_f = singles.tile([P, P], F32)
            nc.gpsimd.iota(
                iota_f, [[1, P]], channel_multiplier=0,
                allow_small_or_imprecise_dtypes=True,
            )
            iota_p = singles.tile([P, 1], F32)
            nc.gpsimd.iota(
                iota_p, [[0, 1]], channel_multiplier=1,
                allow_small_or_imprecise_dtypes=True,
            )
            ident = singles.tile([P, P], F32)
            nc.vector.tensor_scalar(
                out=ident, in0=iota_f, scalar1=iota_p, scalar2=None,
                op0=mybir.AluOpType.is_equal,
            )
            # stride-0 broadcast loads, first in the sync HWDGE ring so the
            # weights land before the first dep tile finishes
            wt = singles.tile([P, 2 * D], F32)
            nc.sync.dma_start(out=wt, in_=w.to_broadcast([P, 2 * D]))
            bt = singles.tile([P, 1], F32)
            nc.sync.dma_start(out=bt, in_=b.to_broadcast([P, 1]))

            def eng(name):
                return {"V": nc.vector, "A": nc.scalar, "G": nc.gpsimd}[name]

            def reduce_to(engine, dst, prod):
                if engine == "A":
                    trash = scratch.tile([P, D], F32, tag="trash")
                    nc.scalar.activation(
                        out=trash,
                        in_=prod,
                        func=mybir.ActivationFunctionType.Copy,
                        accum_out=dst,
                    )
                else:
                    nc.vector.reduce_sum(dst, prod, axis=mybir.AxisListType.X)

            for bi in range(BPC):
                # ---- loads: dep first (it gates the broadcast row) ----
                dep_t = []
                for t in range(NPAIR):
                    dt_ = loads.tile([P, 2, D], F32, tag="dep")
                    nc.sync.dma_start(out=dt_, in_=dep_v[bi, t])
                    dep_t.append(dt_)
                head_t = []
                for t in range(NPAIR):
                    ht = loads.tile([P, 2, D], F32, tag="head")
                    nc.sync.dma_start(out=ht, in_=head_v[bi, t])
                    head_t.append(ht)

                # ---- s_d chunks -> stationary-broadcast matmuls into PSUM ----
                sd = svec.tile([P, C], F32, tag="sd")
                ps = psum.tile([P, S], F32, tag="ps")
                for k in range(C):
                    src = dep_t[k // 2][:, k % 2, :]
                    prod = scratch.tile([P, D], F32, tag="prod")
                    eng(DEP_MULT_ENG[k]).tensor_mul(prod, src, wt[:, D : 2 * D])
                    reduce_to(DEP_RED_ENG[k], sd[:, k : k + 1], prod)
                    nc.tensor.matmul(
                        ps[:, k * P : (k + 1) * P],
                        lhsT=sd[:, k : k + 1].broadcast_to((P, P)),
                        rhs=ident,
                        start=True,
                        stop=True,
                    )
                # one SBUF copy of the broadcast row, with edge_b folded in;
                # DVE-side adds then run in 2x perf mode (SBUF source)
                sdb_sb = bcast.tile([P, S], F32, tag="sdbsb")
                nc.scalar.add(out=sdb_sb, in_=ps, add=bt)

                # ---- s_h chunks + output chunks ----
                sh = svec.tile([P, C], F32, tag="sh")
                for t in range(NPAIR):
                    ot = outs.tile([P, 2, S], F32, tag="ot")
                    for i in range(2):
                        c = 2 * t + i
                        src = head_t[t][:, i, :]
                        prod = scratch.tile([P, D], F32, tag="prod")
                        eng(HEAD_MULT_ENG[c]).tensor_mul(prod, src, wt[:, 0:D])
                        reduce_to(HEAD_RED_ENG[c], sh[:, c : c + 1], prod)
                        if OUT_ENG[c] == "A":
                            nc.scalar.add(
                                out=ot[:, i, :], in_=sdb_sb, add=sh[:, c : c + 1]
                            )
                        else:
                            nc.vector.tensor_scalar_add(
                                ot[:, i, :], sdb_sb, sh[:, c : c + 1]
                            )
                    nc.sync.dma_start(out=out_v[bi, t], in_=ot)
    nc.compile()
    return nc


def kernel(head, dep, edge_W, edge_b, _trace=False):
    nc = build_program()
    in_maps = []
    for k in range(N_CORES):
        in_maps.append(
            {
                "head": np.ascontiguousarray(head[k * BPC : (k + 1) * BPC]),
                "dep": np.ascontiguousarray(dep[k * BPC : (k + 1) * BPC]),
                "edge_W": np.ascontiguousarray(edge_W),
                "edge_b": np.ascontiguousarray(edge_b),
            }
        )
    res = run_bass_kernel_spmd(nc, in_maps, core_ids=list(range(N_CORES)), trace=_trace)
    out = np.concatenate([r["out"] for r in res.results], axis=0)
    if _trace:
        return out, res
    return out


if __name__ == "__main__":
    rng = np.random.default_rng(0)
    head = rng.standard_normal((B, S, D), dtype=np.float32)
    dep = rng.standard_normal((B, S, D), dtype=np.float32)
    edge_W = rng.standard_normal((1, 2 * D), dtype=np.float32)
    edge_b = rng.standard_normal((1,), dtype=np.float32)
    out = kernel(head, dep, edge_W, edge_b)
    ref = (
        head @ edge_W[0, :D]
    )[:, :, None] + (dep @ edge_W[0, D:])[:, None, :] + edge_b[0]
    err = np.abs(out - ref).max() / np.abs(ref).max()
    print("max rel err:", err)
